# revision 5
# baseline (speedup 1.0000x reference)
"""BiMambaHead kernel for 8 Trainium2 NeuronCores.

Strategy: data-parallel over batch (32 seqs -> 4 per core). The dominant
matmul (in_proj, x @ W^T, shared between the forward and backward Mamba
directions) runs on-device as a Bass/Tile SPMD kernel in bf16 (fp32 for the
dt rows), feature-major output. The sequential tail (depthwise conv,
selective scan, gated RMSNorm, fused output projection) runs on host with a
chunked SSD formulation (no per-timestep Python loop).

Hardcoded shapes: B=32, L=1024, D_MODEL=512, D_IN_PROJ=2096.
"""

import numpy as np

D_MODEL = 512
D_INNER = 1024
D_STATE = 16
HEADDIM = 64
NHEADS = 16
D_CONV = 4
NB_CLS = 96
CONV_DIM = D_INNER + 2 * D_STATE          # 1056
D_IN_PROJ = 2 * D_INNER + 2 * D_STATE + NHEADS  # 2096
NF_BF = D_IN_PROJ - NHEADS                 # 2080 features computed in bf16
B, L = 32, 1024
N_CORES = 8
B_PER = B // N_CORES                       # 4 seqs per core
TOK = B_PER * L                            # 4096 tokens per core

_cached = {}


def _build_bass():
    import concourse.bacc as bacc
    import concourse.mybir as mybir
    import concourse.tile as tile

    nc = bacc.Bacc("TRN2", target_bir_lowering=False)
    wt = nc.dram_tensor("wt", [D_MODEL, D_IN_PROJ], mybir.dt.bfloat16,
                        kind="ExternalInput")
    xt = nc.dram_tensor("xt", [D_MODEL, TOK], mybir.dt.bfloat16,
                        kind="ExternalInput")
    zx = nc.dram_tensor("zx", [NF_BF, TOK], mybir.dt.bfloat16,
                        kind="ExternalOutput")
    dtr = nc.dram_tensor("dtr", [NHEADS, TOK], mybir.dt.float32,
                         kind="ExternalOutput")

    KT = D_MODEL // 128                    # 4 k-tiles
    NF = 512                               # token chunk per matmul (psum bank)
    NT = TOK // NF                         # 8 token chunks
    FT = (D_IN_PROJ + 127) // 128          # 17 feature tiles (last = 48 rows)

    with tile.TileContext(nc) as tc:
        with (
            tc.tile_pool(name="w", bufs=1) as wpool,
            tc.tile_pool(name="x", bufs=1) as xpool,
            tc.tile_pool(name="o", bufs=8) as opool,
            tc.tile_pool(name="ps", bufs=8, space="PSUM") as pspool,
        ):
            # w first (every matmul needs it); x streamed in 512-token
            # chunks so the first matmuls start after ~128KB, not ~4MB.
            w_t = [wpool.tile([128, D_IN_PROJ], mybir.dt.bfloat16,
                              name=f"w{k}") for k in range(KT)]
            x_t = [xpool.tile([128, TOK], mybir.dt.bfloat16,
                              name=f"x{k}") for k in range(KT)]
            for k in range(KT):
                nc.sync.dma_start(w_t[k][:], wt[k * 128:(k + 1) * 128, :])
            for t in range(NT):
                for k in range(KT):
                    nc.sync.dma_start(
                        x_t[k][:, t * NF:(t + 1) * NF],
                        xt[k * 128:(k + 1) * 128, t * NF:(t + 1) * NF])

            for t in range(NT):
                for f in range(FT):
                    fm = min(128, D_IN_PROJ - f * 128)
                    ps = pspool.tile([128, NF], mybir.dt.float32)
                    for k in range(KT):
                        nc.tensor.matmul(
                            ps[:fm, :],
                            w_t[k][:, f * 128:f * 128 + fm],
                            x_t[k][:, t * NF:(t + 1) * NF],
                            start=(k == 0), stop=(k == KT - 1),
                        )
                    if f < FT - 1:
                        ot = opool.tile([128, NF], mybir.dt.bfloat16, tag="ot")
                        nc.vector.tensor_copy(ot[:fm, :], ps[:fm, :])
                        nc.sync.dma_start(
                            zx[f * 128:f * 128 + fm, t * NF:(t + 1) * NF],
                            ot[:fm, :])
                    else:
                        # last tile: 32 bf16 rows (B/C) + 16 fp32 rows (dt)
                        ot = opool.tile([128, NF], mybir.dt.bfloat16, tag="ot")
                        nc.vector.tensor_copy(ot[:32, :], ps[:32, :])
                        nc.sync.dma_start(
                            zx[f * 128:f * 128 + 32, t * NF:(t + 1) * NF],
                            ot[:32, :])
                        of = opool.tile([16, NF], mybir.dt.float32, tag="of")
                        nc.scalar.copy(of[:, :], ps[32:48, :])
                        nc.sync.dma_start(
                            dtr[:, t * NF:(t + 1) * NF], of[:, :])
    nc.finalize()
    return nc


def _in_proj_device(x, in_proj_w):
    """x: [B, L, D_MODEL] fp32 -> (zx [B, L, 2080] f32 from bf16,
    dtr [B, L, 16] f32) via 8 cores."""
    from concourse.bass_utils import run_bass_kernel_spmd
    import ml_dtypes

    if "nc" not in _cached:
        _cached["nc"] = _build_bass()
    nc = _cached["nc"]

    if "wt_bf" not in _cached:
        _cached["wt_bf"] = np.ascontiguousarray(
            in_proj_w.T.astype(ml_dtypes.bfloat16))
    wt_bf = _cached["wt_bf"]
    in_maps = []
    for c in range(N_CORES):
        xc = x[c * B_PER:(c + 1) * B_PER].reshape(TOK, D_MODEL)
        xtc = np.ascontiguousarray(xc.T.astype(ml_dtypes.bfloat16))
        in_maps.append({"wt": wt_bf, "xt": xtc})

    res = run_bass_kernel_spmd(nc, in_maps, list(range(N_CORES)))
    _cached["exec_ns"] = getattr(res, "exec_time_ns", None)
    it = getattr(res, "instructions_and_trace", None)
    if it is not None:
        _cached["trace"] = it
    outs = res.results if hasattr(res, "results") else res
    zx = np.empty((B, L, NF_BF), dtype=np.float32)
    dtr = np.empty((B, L, NHEADS), dtype=np.float32)
    for c in range(N_CORES):
        z = np.asarray(outs[c]["zx"], dtype=np.float32)   # [2080, 4096]
        d = np.asarray(outs[c]["dtr"])                     # [16, 4096]
        zx[c * B_PER:(c + 1) * B_PER] = z.T.reshape(B_PER, L, NF_BF)
        dtr[c * B_PER:(c + 1) * B_PER] = d.T.reshape(B_PER, L, NHEADS)
    return zx, dtr


def _softplus(x):
    return np.log1p(np.exp(-np.abs(x))) + np.maximum(x, 0.0)


def _silu(x):
    return x / (1.0 + np.exp(-x))


Q = 32  # scan chunk length


def _scan_chunked(xs, Bm, Cm, dt, a):
    """Chunked SSD selective scan (no per-step loop), batched-matmul layout.
    xs [B,L,H,P], Bm/Cm [B,L,N], dt/a [B,L,H] (a = dt*A, negative).
    Returns y [B,L,H,P]."""
    NC = L // Q
    xs_c = np.ascontiguousarray(
        xs.reshape(B, NC, Q, NHEADS, HEADDIM).transpose(0, 1, 3, 2, 4))
    B_c = Bm.reshape(B, NC, Q, D_STATE)
    C_c = Cm.reshape(B, NC, Q, D_STATE)
    dt_c = dt.reshape(B, NC, Q, NHEADS)
    a_c = a.reshape(B, NC, Q, NHEADS)
    Acum = np.cumsum(a_c, axis=2)                        # [B,NC,Q,H]

    G = np.matmul(C_c, B_c.transpose(0, 1, 3, 2))        # [B,NC,t,s]
    AcumT = Acum.transpose(0, 1, 3, 2)                   # [B,NC,H,t]
    diff = AcumT[:, :, :, :, None] - AcumT[:, :, :, None, :]
    np.minimum(diff, 0.0, out=diff)      # upper triangle clamped (masked next)
    Ldec = np.exp(diff, out=diff)                        # [B,NC,H,t,s]
    mask = np.tril(np.ones((Q, Q), dtype=np.float32))
    M = Ldec * G[:, :, None, :, :]
    M *= mask
    M *= dt_c.transpose(0, 1, 3, 2)[:, :, :, None, :]
    y = np.matmul(M, xs_c)                               # [B,NC,H,t,P]

    Asum = Acum[:, :, -1, :]                             # [B,NC,H]
    w_s = np.exp(Asum[:, :, None, :] - Acum) * dt_c      # [B,NC,s,H]
    Bw = B_c[:, :, :, None, :] * w_s[..., None]          # [B,NC,s,H,N]
    S = np.matmul(Bw.transpose(0, 1, 3, 4, 2), xs_c)     # [B,NC,H,N,P]

    dA_chunk = np.exp(Asum)
    S_run = np.empty((B, NC, NHEADS, D_STATE, HEADDIM), dtype=np.float32)
    S_prev = np.zeros((B, NHEADS, D_STATE, HEADDIM), dtype=np.float32)
    for c in range(NC):
        S_run[:, c] = S_prev
        S_prev = S_prev * dA_chunk[:, c, :, None, None] + S[:, c]

    CE = C_c[:, :, :, None, :] * np.exp(AcumT).transpose(0, 1, 3, 2)[..., None]
    y += np.matmul(CE.transpose(0, 1, 3, 2, 4), S_run)   # [B,NC,H,t,P]
    return y.transpose(0, 1, 3, 2, 4).reshape(B, L, NHEADS, HEADDIM)


def _mamba_tail(xBC, sz, dt, A, conv_w, conv_b, D, flip):
    """xBC [B,L,1056] fp32, sz = silu(z) [B,L,1024] fp32 (shared between
    directions), dt [B,L,H] softplus'd. Returns gated+normed y [B,L,D_INNER]
    (original time order)."""
    if flip:
        xBC_t = np.ascontiguousarray(xBC[:, ::-1])
        dt_t = np.ascontiguousarray(dt[:, ::-1])
    else:
        xBC_t = xBC
        dt_t = dt

    # causal depthwise conv, k=4
    conv = xBC_t * conv_w[:, D_CONV - 1]
    for k in range(D_CONV - 1):
        sh = D_CONV - 1 - k
        conv[:, sh:] += xBC_t[:, :-sh] * conv_w[:, k]
    conv += conv_b
    xBC_c = _silu(conv)

    xs = xBC_c[..., :D_INNER].reshape(B, L, NHEADS, HEADDIM)
    Bm = xBC_c[..., D_INNER:D_INNER + D_STATE]
    Cm = xBC_c[..., D_INNER + D_STATE:]
    a = dt_t * A

    y = _scan_chunked(xs, Bm, Cm, dt_t, a)
    y += xs * D[None, None, :, None]
    y = y.reshape(B, L, D_INNER)
    if flip:
        y = y[:, ::-1]

    y = y * sz
    ss = np.einsum('blc,blc->bl', y, y, optimize=True)[..., None]
    y *= (1.0 / np.sqrt(ss * (1.0 / D_INNER) + 1e-5))
    return y


def kernel(x, in_proj_w, conv_w, conv_b, dt_bias, A_log, D, norm_w,
           out_proj_w, fc_w, fc_b):
    x = np.asarray(x, dtype=np.float32)
    in_proj_w = np.asarray(in_proj_w, dtype=np.float32)
    conv_w = np.asarray(conv_w, dtype=np.float32)
    conv_b = np.asarray(conv_b, dtype=np.float32)
    dt_bias = np.asarray(dt_bias, dtype=np.float32)
    A_log = np.asarray(A_log, dtype=np.float32)
    D = np.asarray(D, dtype=np.float32)
    norm_w = np.asarray(norm_w, dtype=np.float32)
    out_proj_w = np.asarray(out_proj_w, dtype=np.float32)
    fc_w = np.asarray(fc_w, dtype=np.float32)
    fc_b = np.asarray(fc_b, dtype=np.float32)

    try:
        zx, dtr = _in_proj_device(x, in_proj_w)
    except Exception:
        full = (x.reshape(-1, D_MODEL) @ in_proj_w.T).reshape(B, L, D_IN_PROJ)
        zx = full[..., :NF_BF]
        dtr = full[..., NF_BF:]

    dt = _softplus(dtr + dt_bias)
    A = -np.exp(A_log)
    z = zx[..., :D_INNER]
    xBC = np.ascontiguousarray(zx[..., D_INNER:])
    sz = _silu(z)

    y_f = _mamba_tail(xBC, sz, dt, A, conv_w, conv_b, D, False)
    y_b = _mamba_tail(xBC, sz, dt, A, conv_w, conv_b, D, True)
    y_sum = y_f
    y_sum += y_b

    # (out_f + out_b) @ fc^T + b == y_sum @ (fc @ out_proj)^T + b
    wc = ((fc_w @ out_proj_w) * norm_w[None, :]).astype(np.float32)
    out = y_sum.reshape(-1, D_INNER) @ wc.T + fc_b
    return out.reshape(B, L, NB_CLS).astype(np.float32)


# revision 6
# speedup vs baseline: 1.0201x; 1.0201x over previous
"""BiMambaHead kernel for 8 Trainium2 NeuronCores.

Strategy: data-parallel over batch (32 seqs -> 4 per core). The dominant
matmul (in_proj, x @ W^T, shared between the forward and backward Mamba
directions) runs on-device as a Bass/Tile SPMD kernel in bf16 (fp32 for the
dt rows), feature-major output. The sequential tail (depthwise conv,
selective scan, gated RMSNorm, fused output projection) runs on host with a
chunked SSD formulation (no per-timestep Python loop).

Hardcoded shapes: B=32, L=1024, D_MODEL=512, D_IN_PROJ=2096.
"""

import numpy as np

D_MODEL = 512
D_INNER = 1024
D_STATE = 16
HEADDIM = 64
NHEADS = 16
D_CONV = 4
NB_CLS = 96
CONV_DIM = D_INNER + 2 * D_STATE          # 1056
D_IN_PROJ = 2 * D_INNER + 2 * D_STATE + NHEADS  # 2096
NF_BF = D_IN_PROJ - NHEADS                 # 2080 features computed in bf16
B, L = 32, 1024
N_CORES = 8
B_PER = B // N_CORES                       # 4 seqs per core
TOK = B_PER * L                            # 4096 tokens per core

_cached = {}


def _build_bass():
    import concourse.bacc as bacc
    import concourse.mybir as mybir
    import concourse.tile as tile

    nc = bacc.Bacc("TRN2", target_bir_lowering=False)
    wt = nc.dram_tensor("wt", [D_MODEL, D_IN_PROJ], mybir.dt.bfloat16,
                        kind="ExternalInput")
    xt = nc.dram_tensor("xt", [D_MODEL, TOK], mybir.dt.bfloat16,
                        kind="ExternalInput")
    zx = nc.dram_tensor("zx", [NF_BF, TOK], mybir.dt.bfloat16,
                        kind="ExternalOutput")
    dtr = nc.dram_tensor("dtr", [NHEADS, TOK], mybir.dt.float32,
                         kind="ExternalOutput")

    KT = D_MODEL // 128                    # 4 k-tiles
    NF = 512                               # token chunk per matmul (psum bank)
    NT = TOK // NF                         # 8 token chunks
    FT = (D_IN_PROJ + 127) // 128          # 17 feature tiles (last = 48 rows)

    with tile.TileContext(nc) as tc:
        with (
            tc.tile_pool(name="w", bufs=1) as wpool,
            tc.tile_pool(name="x", bufs=1) as xpool,
            tc.tile_pool(name="o", bufs=8) as opool,
            tc.tile_pool(name="ps", bufs=8, space="PSUM") as pspool,
        ):
            # w first (every matmul needs it); x streamed in 512-token
            # chunks so the first matmuls start after ~128KB, not ~4MB.
            w_t = [wpool.tile([128, D_IN_PROJ], mybir.dt.bfloat16,
                              name=f"w{k}") for k in range(KT)]
            x_t = [xpool.tile([128, TOK], mybir.dt.bfloat16,
                              name=f"x{k}") for k in range(KT)]
            for k in range(KT):
                nc.sync.dma_start(w_t[k][:], wt[k * 128:(k + 1) * 128, :])
            half = TOK // 2
            for h in range(2):
                for k in range(KT):
                    nc.sync.dma_start(
                        x_t[k][:, h * half:(h + 1) * half],
                        xt[k * 128:(k + 1) * 128, h * half:(h + 1) * half])

            for t in range(NT):
                for f in range(FT):
                    fm = min(128, D_IN_PROJ - f * 128)
                    ps = pspool.tile([128, NF], mybir.dt.float32)
                    for k in range(KT):
                        nc.tensor.matmul(
                            ps[:fm, :],
                            w_t[k][:, f * 128:f * 128 + fm],
                            x_t[k][:, t * NF:(t + 1) * NF],
                            start=(k == 0), stop=(k == KT - 1),
                        )
                    if f < FT - 1:
                        ot = opool.tile([128, NF], mybir.dt.bfloat16, tag="ot")
                        nc.vector.tensor_copy(ot[:fm, :], ps[:fm, :])
                        nc.sync.dma_start(
                            zx[f * 128:f * 128 + fm, t * NF:(t + 1) * NF],
                            ot[:fm, :])
                    else:
                        # last tile: 32 bf16 rows (B/C) + 16 fp32 rows (dt)
                        ot = opool.tile([128, NF], mybir.dt.bfloat16, tag="ot")
                        nc.vector.tensor_copy(ot[:32, :], ps[:32, :])
                        nc.sync.dma_start(
                            zx[f * 128:f * 128 + 32, t * NF:(t + 1) * NF],
                            ot[:32, :])
                        of = opool.tile([16, NF], mybir.dt.float32, tag="of")
                        nc.scalar.copy(of[:, :], ps[32:48, :])
                        nc.sync.dma_start(
                            dtr[:, t * NF:(t + 1) * NF], of[:, :])
    nc.finalize()
    return nc


def _in_proj_device(x, in_proj_w):
    """x: [B, L, D_MODEL] fp32 -> (zx [B, L, 2080] f32 from bf16,
    dtr [B, L, 16] f32) via 8 cores."""
    from concourse.bass_utils import run_bass_kernel_spmd
    import ml_dtypes

    if "nc" not in _cached:
        _cached["nc"] = _build_bass()
    nc = _cached["nc"]

    if "wt_bf" not in _cached:
        _cached["wt_bf"] = np.ascontiguousarray(
            in_proj_w.T.astype(ml_dtypes.bfloat16))
    wt_bf = _cached["wt_bf"]
    in_maps = []
    for c in range(N_CORES):
        xc = x[c * B_PER:(c + 1) * B_PER].reshape(TOK, D_MODEL)
        xtc = np.ascontiguousarray(xc.T.astype(ml_dtypes.bfloat16))
        in_maps.append({"wt": wt_bf, "xt": xtc})

    res = run_bass_kernel_spmd(nc, in_maps, list(range(N_CORES)))
    _cached["exec_ns"] = getattr(res, "exec_time_ns", None)
    it = getattr(res, "instructions_and_trace", None)
    if it is not None:
        _cached["trace"] = it
    outs = res.results if hasattr(res, "results") else res
    zx = np.empty((B, L, NF_BF), dtype=np.float32)
    dtr = np.empty((B, L, NHEADS), dtype=np.float32)
    for c in range(N_CORES):
        z = np.asarray(outs[c]["zx"], dtype=np.float32)   # [2080, 4096]
        d = np.asarray(outs[c]["dtr"])                     # [16, 4096]
        zx[c * B_PER:(c + 1) * B_PER] = z.T.reshape(B_PER, L, NF_BF)
        dtr[c * B_PER:(c + 1) * B_PER] = d.T.reshape(B_PER, L, NHEADS)
    return zx, dtr


def _softplus(x):
    return np.log1p(np.exp(-np.abs(x))) + np.maximum(x, 0.0)


def _silu(x):
    return x / (1.0 + np.exp(-x))


Q = 32  # scan chunk length


def _scan_chunked(xs, Bm, Cm, dt, a):
    """Chunked SSD selective scan (no per-step loop), batched-matmul layout.
    xs [B,L,H,P], Bm/Cm [B,L,N], dt/a [B,L,H] (a = dt*A, negative).
    Returns y [B,L,H,P]."""
    NC = L // Q
    xs_c = np.ascontiguousarray(
        xs.reshape(B, NC, Q, NHEADS, HEADDIM).transpose(0, 1, 3, 2, 4))
    B_c = Bm.reshape(B, NC, Q, D_STATE)
    C_c = Cm.reshape(B, NC, Q, D_STATE)
    dt_c = dt.reshape(B, NC, Q, NHEADS)
    a_c = a.reshape(B, NC, Q, NHEADS)
    Acum = np.cumsum(a_c, axis=2)                        # [B,NC,Q,H]

    G = np.matmul(C_c, B_c.transpose(0, 1, 3, 2))        # [B,NC,t,s]
    AcumT = Acum.transpose(0, 1, 3, 2)                   # [B,NC,H,t]
    diff = AcumT[:, :, :, :, None] - AcumT[:, :, :, None, :]
    np.minimum(diff, 0.0, out=diff)      # upper triangle clamped (masked next)
    Ldec = np.exp(diff, out=diff)                        # [B,NC,H,t,s]
    mask = np.tril(np.ones((Q, Q), dtype=np.float32))
    M = Ldec * G[:, :, None, :, :]
    M *= mask
    M *= dt_c.transpose(0, 1, 3, 2)[:, :, :, None, :]
    y = np.matmul(M, xs_c)                               # [B,NC,H,t,P]

    Asum = Acum[:, :, -1, :]                             # [B,NC,H]
    w_s = np.exp(Asum[:, :, None, :] - Acum) * dt_c      # [B,NC,s,H]
    Bw = B_c[:, :, :, None, :] * w_s[..., None]          # [B,NC,s,H,N]
    S = np.matmul(Bw.transpose(0, 1, 3, 4, 2), xs_c)     # [B,NC,H,N,P]

    dA_chunk = np.exp(Asum)
    S_run = np.empty((B, NC, NHEADS, D_STATE, HEADDIM), dtype=np.float32)
    S_prev = np.zeros((B, NHEADS, D_STATE, HEADDIM), dtype=np.float32)
    for c in range(NC):
        S_run[:, c] = S_prev
        S_prev = S_prev * dA_chunk[:, c, :, None, None] + S[:, c]

    CE = C_c[:, :, :, None, :] * np.exp(AcumT).transpose(0, 1, 3, 2)[..., None]
    y += np.matmul(CE.transpose(0, 1, 3, 2, 4), S_run)   # [B,NC,H,t,P]
    return y.transpose(0, 1, 3, 2, 4).reshape(B, L, NHEADS, HEADDIM)


def _mamba_tail(xBC, sz, dt, A, conv_w, conv_b, D, flip):
    """xBC [B,L,1056] fp32, sz = silu(z) [B,L,1024] fp32 (shared between
    directions), dt [B,L,H] softplus'd. Returns gated+normed y [B,L,D_INNER]
    (original time order)."""
    if flip:
        xBC_t = np.ascontiguousarray(xBC[:, ::-1])
        dt_t = np.ascontiguousarray(dt[:, ::-1])
    else:
        xBC_t = xBC
        dt_t = dt

    # causal depthwise conv, k=4
    conv = xBC_t * conv_w[:, D_CONV - 1]
    for k in range(D_CONV - 1):
        sh = D_CONV - 1 - k
        conv[:, sh:] += xBC_t[:, :-sh] * conv_w[:, k]
    conv += conv_b
    xBC_c = _silu(conv)

    xs = xBC_c[..., :D_INNER].reshape(B, L, NHEADS, HEADDIM)
    Bm = xBC_c[..., D_INNER:D_INNER + D_STATE]
    Cm = xBC_c[..., D_INNER + D_STATE:]
    a = dt_t * A

    y = _scan_chunked(xs, Bm, Cm, dt_t, a)
    y += xs * D[None, None, :, None]
    y = y.reshape(B, L, D_INNER)
    if flip:
        y = y[:, ::-1]

    y = y * sz
    ss = np.einsum('blc,blc->bl', y, y, optimize=True)[..., None]
    y *= (1.0 / np.sqrt(ss * (1.0 / D_INNER) + 1e-5))
    return y


def kernel(x, in_proj_w, conv_w, conv_b, dt_bias, A_log, D, norm_w,
           out_proj_w, fc_w, fc_b):
    x = np.asarray(x, dtype=np.float32)
    in_proj_w = np.asarray(in_proj_w, dtype=np.float32)
    conv_w = np.asarray(conv_w, dtype=np.float32)
    conv_b = np.asarray(conv_b, dtype=np.float32)
    dt_bias = np.asarray(dt_bias, dtype=np.float32)
    A_log = np.asarray(A_log, dtype=np.float32)
    D = np.asarray(D, dtype=np.float32)
    norm_w = np.asarray(norm_w, dtype=np.float32)
    out_proj_w = np.asarray(out_proj_w, dtype=np.float32)
    fc_w = np.asarray(fc_w, dtype=np.float32)
    fc_b = np.asarray(fc_b, dtype=np.float32)

    try:
        zx, dtr = _in_proj_device(x, in_proj_w)
    except Exception:
        full = (x.reshape(-1, D_MODEL) @ in_proj_w.T).reshape(B, L, D_IN_PROJ)
        zx = full[..., :NF_BF]
        dtr = full[..., NF_BF:]

    dt = _softplus(dtr + dt_bias)
    A = -np.exp(A_log)
    z = zx[..., :D_INNER]
    xBC = np.ascontiguousarray(zx[..., D_INNER:])
    sz = _silu(z)

    y_f = _mamba_tail(xBC, sz, dt, A, conv_w, conv_b, D, False)
    y_b = _mamba_tail(xBC, sz, dt, A, conv_w, conv_b, D, True)
    y_sum = y_f
    y_sum += y_b

    # (out_f + out_b) @ fc^T + b == y_sum @ (fc @ out_proj)^T + b
    wc = ((fc_w @ out_proj_w) * norm_w[None, :]).astype(np.float32)
    out = y_sum.reshape(-1, D_INNER) @ wc.T + fc_b
    return out.reshape(B, L, NB_CLS).astype(np.float32)


# revision 8
# speedup vs baseline: 1.0277x; 1.0074x over previous
"""BiMambaHead kernel for 8 Trainium2 NeuronCores.

Strategy: data-parallel over batch (32 seqs -> 4 per core). The dominant
matmul (in_proj, x @ W^T, shared between the forward and backward Mamba
directions) runs on-device as a Bass/Tile SPMD kernel in bf16 (fp32 for the
dt rows), feature-major output. The sequential tail (depthwise conv,
selective scan, gated RMSNorm, fused output projection) runs on host with a
chunked SSD formulation (no per-timestep Python loop).

Hardcoded shapes: B=32, L=1024, D_MODEL=512, D_IN_PROJ=2096.
"""

import numpy as np

D_MODEL = 512
D_INNER = 1024
D_STATE = 16
HEADDIM = 64
NHEADS = 16
D_CONV = 4
NB_CLS = 96
CONV_DIM = D_INNER + 2 * D_STATE          # 1056
D_IN_PROJ = 2 * D_INNER + 2 * D_STATE + NHEADS  # 2096
NF_BF = D_IN_PROJ - NHEADS                 # 2080 features computed in bf16
B, L = 32, 1024
N_CORES = 8
B_PER = B // N_CORES                       # 4 seqs per core
TOK = B_PER * L                            # 4096 tokens per core

_cached = {}


def _build_bass():
    import concourse.bacc as bacc
    import concourse.mybir as mybir
    import concourse.tile as tile

    nc = bacc.Bacc("TRN2", target_bir_lowering=False)
    wt = nc.dram_tensor("wt", [D_MODEL, D_IN_PROJ], mybir.dt.bfloat16,
                        kind="ExternalInput")
    xt = nc.dram_tensor("xt", [D_MODEL, TOK], mybir.dt.bfloat16,
                        kind="ExternalInput")
    zx = nc.dram_tensor("zx", [NF_BF, TOK], mybir.dt.bfloat16,
                        kind="ExternalOutput")
    dtr = nc.dram_tensor("dtr", [NHEADS, TOK], mybir.dt.float32,
                         kind="ExternalOutput")

    KT = D_MODEL // 128                    # 4 k-tiles
    NF = 512                               # token chunk per matmul (psum bank)
    NT = TOK // NF                         # 8 token chunks
    FT = (D_IN_PROJ + 127) // 128          # 17 feature tiles (last = 48 rows)

    with tile.TileContext(nc) as tc:
        with (
            tc.tile_pool(name="w", bufs=1) as wpool,
            tc.tile_pool(name="x", bufs=1) as xpool,
            tc.tile_pool(name="o", bufs=12) as opool,
            tc.tile_pool(name="ps", bufs=8, space="PSUM") as pspool,
        ):
            # w first (every matmul needs it); x streamed in 512-token
            # chunks so the first matmuls start after ~128KB, not ~4MB.
            w_t = [wpool.tile([128, D_IN_PROJ], mybir.dt.bfloat16,
                              name=f"w{k}") for k in range(KT)]
            x_t = [xpool.tile([128, TOK], mybir.dt.bfloat16,
                              name=f"x{k}") for k in range(KT)]
            for k in range(KT):
                nc.sync.dma_start(w_t[k][:], wt[k * 128:(k + 1) * 128, :])
            half = TOK // 2
            for h in range(2):
                for k in range(KT):
                    nc.sync.dma_start(
                        x_t[k][:, h * half:(h + 1) * half],
                        xt[k * 128:(k + 1) * 128, h * half:(h + 1) * half])

            for t in range(NT):
                for f in range(FT):
                    fm = min(128, D_IN_PROJ - f * 128)
                    ps = pspool.tile([128, NF], mybir.dt.float32)
                    for k in range(KT):
                        nc.tensor.matmul(
                            ps[:fm, :],
                            w_t[k][:, f * 128:f * 128 + fm],
                            x_t[k][:, t * NF:(t + 1) * NF],
                            start=(k == 0), stop=(k == KT - 1),
                        )
                    if f < FT - 1:
                        ot = opool.tile([128, NF], mybir.dt.bfloat16, tag="ot")
                        nc.vector.tensor_copy(ot[:fm, :], ps[:fm, :])
                        nc.sync.dma_start(
                            zx[f * 128:f * 128 + fm, t * NF:(t + 1) * NF],
                            ot[:fm, :])
                    else:
                        # last tile: 32 bf16 rows (B/C) + 16 fp32 rows (dt)
                        ot = opool.tile([128, NF], mybir.dt.bfloat16, tag="ot")
                        nc.vector.tensor_copy(ot[:32, :], ps[:32, :])
                        nc.sync.dma_start(
                            zx[f * 128:f * 128 + 32, t * NF:(t + 1) * NF],
                            ot[:32, :])
                        of = opool.tile([16, NF], mybir.dt.float32, tag="of")
                        nc.vector.tensor_copy(of[:, :], ps[32:48, :])
                        nc.sync.dma_start(
                            dtr[:, t * NF:(t + 1) * NF], of[:, :])
    nc.finalize()
    return nc


def _in_proj_device(x, in_proj_w):
    """x: [B, L, D_MODEL] fp32 -> (zx [B, L, 2080] f32 from bf16,
    dtr [B, L, 16] f32) via 8 cores."""
    from concourse.bass_utils import run_bass_kernel_spmd
    import ml_dtypes

    if "nc" not in _cached:
        _cached["nc"] = _build_bass()
    nc = _cached["nc"]

    if "wt_bf" not in _cached:
        _cached["wt_bf"] = np.ascontiguousarray(
            in_proj_w.T.astype(ml_dtypes.bfloat16))
    wt_bf = _cached["wt_bf"]
    in_maps = []
    for c in range(N_CORES):
        xc = x[c * B_PER:(c + 1) * B_PER].reshape(TOK, D_MODEL)
        xtc = np.ascontiguousarray(xc.T.astype(ml_dtypes.bfloat16))
        in_maps.append({"wt": wt_bf, "xt": xtc})

    res = run_bass_kernel_spmd(nc, in_maps, list(range(N_CORES)))
    _cached["exec_ns"] = getattr(res, "exec_time_ns", None)
    it = getattr(res, "instructions_and_trace", None)
    if it is not None:
        _cached["trace"] = it
    outs = res.results if hasattr(res, "results") else res
    zx = np.empty((B, L, NF_BF), dtype=np.float32)
    dtr = np.empty((B, L, NHEADS), dtype=np.float32)
    for c in range(N_CORES):
        z = np.asarray(outs[c]["zx"], dtype=np.float32)   # [2080, 4096]
        d = np.asarray(outs[c]["dtr"])                     # [16, 4096]
        zx[c * B_PER:(c + 1) * B_PER] = z.T.reshape(B_PER, L, NF_BF)
        dtr[c * B_PER:(c + 1) * B_PER] = d.T.reshape(B_PER, L, NHEADS)
    return zx, dtr


def _softplus(x):
    return np.log1p(np.exp(-np.abs(x))) + np.maximum(x, 0.0)


def _silu(x):
    return x / (1.0 + np.exp(-x))


Q = 32  # scan chunk length


def _scan_chunked(xs, Bm, Cm, dt, a):
    """Chunked SSD selective scan (no per-step loop), batched-matmul layout.
    xs [B,L,H,P], Bm/Cm [B,L,N], dt/a [B,L,H] (a = dt*A, negative).
    Returns y [B,L,H,P]."""
    NC = L // Q
    xs_c = np.ascontiguousarray(
        xs.reshape(B, NC, Q, NHEADS, HEADDIM).transpose(0, 1, 3, 2, 4))
    B_c = Bm.reshape(B, NC, Q, D_STATE)
    C_c = Cm.reshape(B, NC, Q, D_STATE)
    dt_c = dt.reshape(B, NC, Q, NHEADS)
    a_c = a.reshape(B, NC, Q, NHEADS)
    Acum = np.cumsum(a_c, axis=2)                        # [B,NC,Q,H]

    G = np.matmul(C_c, B_c.transpose(0, 1, 3, 2))        # [B,NC,t,s]
    AcumT = Acum.transpose(0, 1, 3, 2)                   # [B,NC,H,t]
    diff = AcumT[:, :, :, :, None] - AcumT[:, :, :, None, :]
    np.minimum(diff, 0.0, out=diff)      # upper triangle clamped (masked next)
    Ldec = np.exp(diff, out=diff)                        # [B,NC,H,t,s]
    mask = np.tril(np.ones((Q, Q), dtype=np.float32))
    M = Ldec * G[:, :, None, :, :]
    M *= mask
    M *= dt_c.transpose(0, 1, 3, 2)[:, :, :, None, :]
    y = np.matmul(M, xs_c)                               # [B,NC,H,t,P]

    Asum = Acum[:, :, -1, :]                             # [B,NC,H]
    w_s = np.exp(Asum[:, :, None, :] - Acum) * dt_c      # [B,NC,s,H]
    Bw = B_c[:, :, :, None, :] * w_s[..., None]          # [B,NC,s,H,N]
    S = np.matmul(Bw.transpose(0, 1, 3, 4, 2), xs_c)     # [B,NC,H,N,P]

    dA_chunk = np.exp(Asum)
    S_run = np.empty((B, NC, NHEADS, D_STATE, HEADDIM), dtype=np.float32)
    S_prev = np.zeros((B, NHEADS, D_STATE, HEADDIM), dtype=np.float32)
    for c in range(NC):
        S_run[:, c] = S_prev
        S_prev = S_prev * dA_chunk[:, c, :, None, None] + S[:, c]

    CE = C_c[:, :, :, None, :] * np.exp(AcumT).transpose(0, 1, 3, 2)[..., None]
    y += np.matmul(CE.transpose(0, 1, 3, 2, 4), S_run)   # [B,NC,H,t,P]
    return y.transpose(0, 1, 3, 2, 4).reshape(B, L, NHEADS, HEADDIM)


def _mamba_tail(xBC, sz, dt, A, conv_w, conv_b, D, flip):
    """xBC [B,L,1056] fp32, sz = silu(z) [B,L,1024] fp32 (shared between
    directions), dt [B,L,H] softplus'd. Returns gated+normed y [B,L,D_INNER]
    (original time order)."""
    if flip:
        xBC_t = np.ascontiguousarray(xBC[:, ::-1])
        dt_t = np.ascontiguousarray(dt[:, ::-1])
    else:
        xBC_t = xBC
        dt_t = dt

    # causal depthwise conv, k=4
    conv = xBC_t * conv_w[:, D_CONV - 1]
    for k in range(D_CONV - 1):
        sh = D_CONV - 1 - k
        conv[:, sh:] += xBC_t[:, :-sh] * conv_w[:, k]
    conv += conv_b
    xBC_c = _silu(conv)

    xs = xBC_c[..., :D_INNER].reshape(B, L, NHEADS, HEADDIM)
    Bm = xBC_c[..., D_INNER:D_INNER + D_STATE]
    Cm = xBC_c[..., D_INNER + D_STATE:]
    a = dt_t * A

    y = _scan_chunked(xs, Bm, Cm, dt_t, a)
    y += xs * D[None, None, :, None]
    y = y.reshape(B, L, D_INNER)
    if flip:
        y = y[:, ::-1]

    y = y * sz
    ss = np.einsum('blc,blc->bl', y, y, optimize=True)[..., None]
    y *= (1.0 / np.sqrt(ss * (1.0 / D_INNER) + 1e-5))
    return y


def kernel(x, in_proj_w, conv_w, conv_b, dt_bias, A_log, D, norm_w,
           out_proj_w, fc_w, fc_b):
    x = np.asarray(x, dtype=np.float32)
    in_proj_w = np.asarray(in_proj_w, dtype=np.float32)
    conv_w = np.asarray(conv_w, dtype=np.float32)
    conv_b = np.asarray(conv_b, dtype=np.float32)
    dt_bias = np.asarray(dt_bias, dtype=np.float32)
    A_log = np.asarray(A_log, dtype=np.float32)
    D = np.asarray(D, dtype=np.float32)
    norm_w = np.asarray(norm_w, dtype=np.float32)
    out_proj_w = np.asarray(out_proj_w, dtype=np.float32)
    fc_w = np.asarray(fc_w, dtype=np.float32)
    fc_b = np.asarray(fc_b, dtype=np.float32)

    try:
        zx, dtr = _in_proj_device(x, in_proj_w)
    except Exception:
        full = (x.reshape(-1, D_MODEL) @ in_proj_w.T).reshape(B, L, D_IN_PROJ)
        zx = full[..., :NF_BF]
        dtr = full[..., NF_BF:]

    dt = _softplus(dtr + dt_bias)
    A = -np.exp(A_log)
    z = zx[..., :D_INNER]
    xBC = np.ascontiguousarray(zx[..., D_INNER:])
    sz = _silu(z)

    y_f = _mamba_tail(xBC, sz, dt, A, conv_w, conv_b, D, False)
    y_b = _mamba_tail(xBC, sz, dt, A, conv_w, conv_b, D, True)
    y_sum = y_f
    y_sum += y_b

    # (out_f + out_b) @ fc^T + b == y_sum @ (fc @ out_proj)^T + b
    wc = ((fc_w @ out_proj_w) * norm_w[None, :]).astype(np.float32)
    out = y_sum.reshape(-1, D_INNER) @ wc.T + fc_b
    return out.reshape(B, L, NB_CLS).astype(np.float32)
